# revision 42
# baseline (speedup 1.0000x reference)
"""Block-Circulant-Matrix Linear kernel for Trainium2 (8 NeuronCores, SPMD).

Reference computation:
    W[r*64+i, q*64+j] = w[r, q, (i-j) % 64]        (dense 1024x1024 from w[16,16,64])
    y = x @ W.T                                    (x: [32768, 1024] f32)

Strategy (data-parallel over tokens, 4096 tokens/core):
  - Host precomputes the dense W.T (fp16, [in=1024, out=1024]) from the tiny
    compressed w, and the transposed activation x.T (fp16) so the device does
    ZERO transposes and ZERO weight restructuring: TensorE runs nothing but
    the 512 N=512 matmuls per core (the streaming-rate floor, ~1 cycle/row).
  - Per 128-token group g: psum[t128, o1024] += sum_c xt_c[:, g].T @ wt_c
    (c = 8 contraction chunks of 128 in-channels).  c-major ordering over a
    4-group slab keeps 8 PSUM banks rotating and lets matmuls start as soon
    as the first (W chunk, x chunk) DMA pair lands instead of waiting for the
    whole activation load.
  - PSUM -> SBUF drains split across VectorE (low half) and ScalarE (high
    half); y is stored fp16 and upcast on host (halves output DMA traffic).
    y-store DMAs alternate between the scalar and sync HWDGE rings and are
    emitted after each slab's drains so a waiting store never blocks a PSUM
    drain in an engine FIFO (which would stall PSUM recycling and the PE).
  - A warmup matmul spin keeps the PE busy while the first DMAs land so the
    HAM clock-gate is released (2.4 GHz) by the time real work arrives.
"""

import numpy as np

N_CORES = 8
N_TOKENS = 32768
TOK_PER_CORE = N_TOKENS // N_CORES  # 4096
IN_CH = 1024
OUT_CH = 1024
BS = 64
R = OUT_CH // BS  # 16
Q = IN_CH // BS   # 16
KCH = IN_CH // 128   # 8 contraction chunks of 128 partitions
SLAB = 512           # tokens per slab (4 groups of 128)
GRP = 128            # tokens per psum group

_CACHE = {}


def build_nc(tok_per_core=TOK_PER_CORE):
    from contextlib import ExitStack

    import concourse.mybir as mybir
    import concourse.tile as tile
    from concourse import bacc

    f16 = mybir.dt.float16
    f32 = mybir.dt.float32

    n_slabs = tok_per_core // SLAB
    g_per_slab = SLAB // GRP  # 4

    nc = bacc.Bacc("TRN2", target_bir_lowering=False, debug=False)
    xt = nc.dram_tensor("xt", [IN_CH, tok_per_core], f16, kind="ExternalInput").ap()
    wt = nc.dram_tensor("wt", [IN_CH, OUT_CH], f16, kind="ExternalInput").ap()
    y = nc.dram_tensor("y", [tok_per_core, OUT_CH], f16, kind="ExternalOutput").ap()

    with tile.TileContext(nc) as tc, ExitStack() as ctx:
        const_pool = ctx.enter_context(tc.tile_pool(name="const", bufs=1))
        w_pool = ctx.enter_context(tc.tile_pool(name="w", bufs=1))
        x_pool = ctx.enter_context(tc.tile_pool(name="x", bufs=1))
        y_pool = ctx.enter_context(tc.tile_pool(name="y", bufs=8))
        ps_pool = ctx.enter_context(tc.tile_pool(name="ps", bufs=8, space="PSUM"))

        # --- PE warmup: matmuls with no DMA deps keep the PE busy from t=0 so
        # the HAM throttle is released before real matmuls arrive.  The
        # operand content is irrelevant (a zeroed tile); results are never
        # read.  No gpsimd anywhere in the kernel.
        wu = const_pool.tile([128, 128], f16)
        nc.vector.memset(wu, 0.0)
        ps_warm = ps_pool.tile([128, 512], f32, name="ps_warm", tag="ps")
        for i in range(40):
            nc.tensor.matmul(
                ps_warm[:, 0:128], lhsT=wu, rhs=wu, start=True, stop=True
            )

        # --- weight chunks (scalar/ACT ring).  No sub-chunk splitting: DMA
        # completions pace at ~1 ring-position per 0.8us regardless of size,
        # so whole 256KB chunks deliver the most data per position. ---
        w_tiles = []
        with tc.high_priority():
            for c in range(KCH):
                w_c = w_pool.tile([128, OUT_CH], f16, name=f"w_{c}")
                nc.scalar.dma_start(w_c, wt[c * 128 : (c + 1) * 128, :])
                w_tiles.append(w_c)

        # x loads ride the sync/SP HWDGE ring in 256KB super-slab tiles (2KB
        # per partition -> ~2x the DMA efficiency of 1KB descriptors), emitted
        # ~1 super-slab ahead of use so the HWDGE semaphore-lane round-robin
        # stays aligned with time (emitting everything upfront makes later
        # DMAs wait on lane predecessors many slabs in the future).
        SS = 2 * SLAB  # super-slab: 1024 tokens
        x_tiles = {}

        def emit_x_dmas(sp):
            for c in range(KCH):
                x_cs = x_pool.tile([128, SS], f16, name=f"x_{c}_{sp}")
                src = xt[c * 128 : (c + 1) * 128, sp * SS : (sp + 1) * SS]
                nc.sync.dma_start(x_cs, src)
                x_tiles[(c, sp)] = x_cs

        # only super-slab 0 upfront: a smaller t=0 DMA burst reduces the
        # HBM-contention spike from all 8 cores starting simultaneously,
        # which is what sets first-data latency (and its device variance)
        emit_x_dmas(0)

        # --- main loop: c-major accumulation over the slab's live psum groups
        # (each a [128, 1024] 2-bank tile; one N=1024 matmul per (c, g)).
        # The last 512 tokens run as two 2-group mini-slabs so the final
        # drain+store chain after the last matmul is half as long. ---
        n_groups = tok_per_core // GRP
        slab_specs = [(i * 4, 4) for i in range(n_groups // 4 - 1)]
        slab_specs += [(n_groups - 4, 2), (n_groups - 2, 1), (n_groups - 1, 1)]
        for si, (g0, ng) in enumerate(slab_specs):
            if g0 % 8 == 0 and g0 // 8 + 1 < tok_per_core // SS:
                emit_x_dmas(g0 // 8 + 1)
            sp, base = g0 // 8, (g0 % 8) * GRP
            ps = [
                [
                    ps_pool.tile([128, 512], f32, name=f"ps_{g0}_{j}_{h}", tag="ps")
                    for h in range(2)
                ]
                for j in range(ng)
            ]
            for c in range(KCH):
                x_cs = x_tiles[(c, sp)]
                for j in range(ng):
                    col = base + j * GRP
                    for h in range(2):
                        nc.tensor.matmul(
                            ps[j][h],
                            lhsT=x_cs[:, col : col + GRP],
                            rhs=w_tiles[c][:, h * 512 : (h + 1) * 512],
                            start=(c == 0),
                            stop=(c == KCH - 1),
                        )
            # drains split DVE (h0) / ACT (h1); y stores only after the slab's
            # drains so no store ever sits in the ACT FIFO ahead of a PSUM
            # drain, alternating rings (both idle enough)
            y_sbs = []
            for j in range(ng):
                y_sb = y_pool.tile([128, OUT_CH], f16, name=f"y_sb_{g0}_{j}", tag="y")
                nc.vector.tensor_copy(y_sb[:, 0:512], ps[j][0])
                nc.scalar.copy(y_sb[:, 512:1024], ps[j][1])
                y_sbs.append(y_sb)
            last = si == len(slab_specs) - 1
            for j in range(ng):
                row = (g0 + j) * GRP
                if last:
                    # final mini-slab: store halves on both rings in parallel
                    nc.scalar.dma_start(y[row : row + GRP, 0:512], y_sbs[j][:, 0:512])
                    nc.sync.dma_start(
                        y[row : row + GRP, 512:1024], y_sbs[j][:, 512:1024]
                    )
                else:
                    eng = nc.scalar if j % 2 == 0 else nc.sync
                    eng.dma_start(y[row : row + GRP, :], y_sbs[j])

    nc.compile()
    return nc


def get_nc(tok_per_core=TOK_PER_CORE):
    if tok_per_core not in _CACHE:
        _CACHE[tok_per_core] = build_nc(tok_per_core)
    return _CACHE[tok_per_core]


def _build_wt(w):
    """Dense W.T ([in, out], fp16) from compressed w [R, Q, BS]."""
    i = np.arange(BS)
    idx = (i[:, None] - i[None, :]) % BS            # (i, j) -> (i-j) % BS
    Wb = w[:, :, idx]                               # [R, Q, BS(i), BS(j)]
    W = Wb.transpose(0, 2, 1, 3).reshape(R * BS, Q * BS)  # [out, in]
    return W.T.astype(np.float16)                   # [in, out], C-contiguous


def kernel(x: np.ndarray, w: np.ndarray) -> np.ndarray:
    from concourse.bass_utils import run_bass_kernel_spmd

    x = np.asarray(x, dtype=np.float32)
    w = np.asarray(w, dtype=np.float32)
    assert x.shape == (N_TOKENS, IN_CH), x.shape
    assert w.shape == (R, Q, BS), w.shape

    xt_full = x.T.astype(np.float16)                # [IN_CH, N_TOKENS], C-contig
    wt = _build_wt(w)

    nc = get_nc()
    in_maps = [
        {
            "xt": np.ascontiguousarray(
                xt_full[:, i * TOK_PER_CORE : (i + 1) * TOK_PER_CORE]
            ),
            "wt": wt,
        }
        for i in range(N_CORES)
    ]
    res = run_bass_kernel_spmd(nc, in_maps, core_ids=list(range(N_CORES)))
    return np.concatenate([r["y"] for r in res.results], axis=0).astype(np.float32)


# revision 43
# speedup vs baseline: 1.1740x; 1.1740x over previous
"""Block-Circulant-Matrix Linear kernel for Trainium2 (8 NeuronCores, SPMD).

Reference computation:
    W[r*64+i, q*64+j] = w[r, q, (i-j) % 64]        (dense 1024x1024 from w[16,16,64])
    y = x @ W.T                                    (x: [32768, 1024] f32)

Strategy (data-parallel over tokens, 4096 tokens/core):
  - Host precomputes the dense W.T (fp16, [in=1024, out=1024]) from the tiny
    compressed w, and the transposed activation x.T (fp16) so the device does
    ZERO transposes and ZERO weight restructuring: TensorE runs nothing but
    the 512 N=512 matmuls per core (the streaming-rate floor, ~1 cycle/row).
  - Per 128-token group g: psum[t128, o1024] += sum_c xt_c[:, g].T @ wt_c
    (c = 8 contraction chunks of 128 in-channels).  c-major ordering over a
    4-group slab keeps 8 PSUM banks rotating and lets matmuls start as soon
    as the first (W chunk, x chunk) DMA pair lands instead of waiting for the
    whole activation load.
  - PSUM -> SBUF drains split across VectorE (low half) and ScalarE (high
    half); y is stored fp16 and upcast on host (halves output DMA traffic).
    y-store DMAs alternate between the scalar and sync HWDGE rings and are
    emitted after each slab's drains so a waiting store never blocks a PSUM
    drain in an engine FIFO (which would stall PSUM recycling and the PE).
  - A warmup matmul spin keeps the PE busy while the first DMAs land so the
    HAM clock-gate is released (2.4 GHz) by the time real work arrives.
"""

import numpy as np

N_CORES = 8
N_TOKENS = 32768
TOK_PER_CORE = N_TOKENS // N_CORES  # 4096
IN_CH = 1024
OUT_CH = 1024
BS = 64
R = OUT_CH // BS  # 16
Q = IN_CH // BS   # 16
KCH = IN_CH // 128   # 8 contraction chunks of 128 partitions
SLAB = 512           # tokens per slab (4 groups of 128)
GRP = 128            # tokens per psum group

_CACHE = {}


def build_nc(tok_per_core=TOK_PER_CORE):
    from contextlib import ExitStack

    import concourse.mybir as mybir
    import concourse.tile as tile
    from concourse import bacc

    f16 = mybir.dt.float16
    f32 = mybir.dt.float32

    n_slabs = tok_per_core // SLAB
    g_per_slab = SLAB // GRP  # 4

    nc = bacc.Bacc("TRN2", target_bir_lowering=False, debug=False)
    xt = nc.dram_tensor("xt", [IN_CH, tok_per_core], f16, kind="ExternalInput").ap()
    wt = nc.dram_tensor("wt", [IN_CH, OUT_CH], f16, kind="ExternalInput").ap()
    y = nc.dram_tensor("y", [tok_per_core, OUT_CH], f16, kind="ExternalOutput").ap()

    with tile.TileContext(nc) as tc, ExitStack() as ctx:
        const_pool = ctx.enter_context(tc.tile_pool(name="const", bufs=1))
        w_pool = ctx.enter_context(tc.tile_pool(name="w", bufs=1))
        x_pool = ctx.enter_context(tc.tile_pool(name="x", bufs=1))
        y_pool = ctx.enter_context(tc.tile_pool(name="y", bufs=8))
        ps_pool = ctx.enter_context(tc.tile_pool(name="ps", bufs=8, space="PSUM"))

        # --- PE warmup: matmuls with no DMA deps keep the PE busy from t=0 so
        # the HAM throttle is released before real matmuls arrive.  The
        # operand content is irrelevant (a zeroed tile); results are never
        # read.  No gpsimd anywhere in the kernel.
        wu = const_pool.tile([128, 128], f16)
        nc.vector.memset(wu, 0.0)
        ps_warm = ps_pool.tile([128, 512], f32, name="ps_warm", tag="ps")
        for i in range(40):
            nc.tensor.matmul(
                ps_warm[:, 0:128], lhsT=wu, rhs=wu, start=True, stop=True
            )

        # --- weight chunks (scalar/ACT ring).  No sub-chunk splitting: DMA
        # completions pace at ~1 ring-position per 0.8us regardless of size,
        # so whole 256KB chunks deliver the most data per position. ---
        w_tiles = []
        with tc.high_priority():
            for c in range(KCH):
                w_c = w_pool.tile([128, OUT_CH], f16, name=f"w_{c}")
                nc.scalar.dma_start(w_c, wt[c * 128 : (c + 1) * 128, :])
                w_tiles.append(w_c)

        # x loads ride the sync/SP HWDGE ring in 256KB super-slab tiles (2KB
        # per partition -> ~2x the DMA efficiency of 1KB descriptors), emitted
        # ~1 super-slab ahead of use so the HWDGE semaphore-lane round-robin
        # stays aligned with time (emitting everything upfront makes later
        # DMAs wait on lane predecessors many slabs in the future).
        SS = 2 * SLAB  # super-slab: 1024 tokens
        x_tiles = {}

        def emit_x_dmas(sp):
            for c in range(KCH):
                x_cs = x_pool.tile([128, SS], f16, name=f"x_{c}_{sp}")
                src = xt[c * 128 : (c + 1) * 128, sp * SS : (sp + 1) * SS]
                nc.sync.dma_start(x_cs, src)
                x_tiles[(c, sp)] = x_cs

        # only super-slab 0 upfront: a smaller t=0 DMA burst reduces the
        # HBM-contention spike from all 8 cores starting simultaneously,
        # which is what sets first-data latency (and its device variance)
        emit_x_dmas(0)

        # --- main loop: c-major accumulation over the slab's live psum groups
        # (each a [128, 1024] 2-bank tile; one N=1024 matmul per (c, g)).
        # The last 512 tokens run as two 2-group mini-slabs so the final
        # drain+store chain after the last matmul is half as long. ---
        n_groups = tok_per_core // GRP
        slab_specs = [(i * 4, 4) for i in range(n_groups // 4 - 1)]
        slab_specs += [(n_groups - 4, 2), (n_groups - 2, 2)]
        for si, (g0, ng) in enumerate(slab_specs):
            if g0 % 8 == 0 and g0 // 8 + 1 < tok_per_core // SS:
                emit_x_dmas(g0 // 8 + 1)
            sp, base = g0 // 8, (g0 % 8) * GRP
            ps = [
                [
                    ps_pool.tile([128, 512], f32, name=f"ps_{g0}_{j}_{h}", tag="ps")
                    for h in range(2)
                ]
                for j in range(ng)
            ]
            for c in range(KCH):
                x_cs = x_tiles[(c, sp)]
                for j in range(ng):
                    col = base + j * GRP
                    for h in range(2):
                        nc.tensor.matmul(
                            ps[j][h],
                            lhsT=x_cs[:, col : col + GRP],
                            rhs=w_tiles[c][:, h * 512 : (h + 1) * 512],
                            start=(c == 0),
                            stop=(c == KCH - 1),
                        )
            # drains split DVE (h0) / ACT (h1); y stores only after the slab's
            # drains so no store ever sits in the ACT FIFO ahead of a PSUM
            # drain, alternating rings (both idle enough)
            y_sbs = []
            for j in range(ng):
                y_sb = y_pool.tile([128, OUT_CH], f16, name=f"y_sb_{g0}_{j}", tag="y")
                nc.vector.tensor_copy(y_sb[:, 0:512], ps[j][0])
                nc.scalar.copy(y_sb[:, 512:1024], ps[j][1])
                y_sbs.append(y_sb)
            last = si == len(slab_specs) - 1
            for j in range(ng):
                row = (g0 + j) * GRP
                if last:
                    # final mini-slab: store halves on both rings in parallel
                    nc.scalar.dma_start(y[row : row + GRP, 0:512], y_sbs[j][:, 0:512])
                    nc.sync.dma_start(
                        y[row : row + GRP, 512:1024], y_sbs[j][:, 512:1024]
                    )
                else:
                    eng = nc.scalar if j % 2 == 0 else nc.sync
                    eng.dma_start(y[row : row + GRP, :], y_sbs[j])

    nc.compile()
    return nc


def get_nc(tok_per_core=TOK_PER_CORE):
    if tok_per_core not in _CACHE:
        _CACHE[tok_per_core] = build_nc(tok_per_core)
    return _CACHE[tok_per_core]


def _build_wt(w):
    """Dense W.T ([in, out], fp16) from compressed w [R, Q, BS]."""
    i = np.arange(BS)
    idx = (i[:, None] - i[None, :]) % BS            # (i, j) -> (i-j) % BS
    Wb = w[:, :, idx]                               # [R, Q, BS(i), BS(j)]
    W = Wb.transpose(0, 2, 1, 3).reshape(R * BS, Q * BS)  # [out, in]
    return W.T.astype(np.float16)                   # [in, out], C-contiguous


def kernel(x: np.ndarray, w: np.ndarray) -> np.ndarray:
    from concourse.bass_utils import run_bass_kernel_spmd

    x = np.asarray(x, dtype=np.float32)
    w = np.asarray(w, dtype=np.float32)
    assert x.shape == (N_TOKENS, IN_CH), x.shape
    assert w.shape == (R, Q, BS), w.shape

    xt_full = x.T.astype(np.float16)                # [IN_CH, N_TOKENS], C-contig
    wt = _build_wt(w)

    nc = get_nc()
    in_maps = [
        {
            "xt": np.ascontiguousarray(
                xt_full[:, i * TOK_PER_CORE : (i + 1) * TOK_PER_CORE]
            ),
            "wt": wt,
        }
        for i in range(N_CORES)
    ]
    res = run_bass_kernel_spmd(nc, in_maps, core_ids=list(range(N_CORES)))
    return np.concatenate([r["y"] for r in res.results], axis=0).astype(np.float32)
